# revision 5
# baseline (speedup 1.0000x reference)
"""Trainium2 Bass kernel for per-anchor local cross-attention.

Problem shapes (hardcoded per contract):
  anchor_tokens   [B=2, V=2048, D=512]
  neighbor_tokens [B=2, V=2048, K=32, D=512]
  Wq/Wk/Wv/Wo     [512, 512], bq/bk/bv/bo [512]
  out             [B, V, D] = attention(anchor over its K neighbors) @ Wo.T + bo

Sharding: V split across 8 cores (attention is local per anchor; weights
replicated). Each core handles Vs = 256 anchors for both batch entries.

Per-core plan (all on one NeuronCore, SPMD over 8):
  - Load anchor / neighbor token tiles naturally ([token, din]); PE-transpose
    128x128 blocks to get [din, token] stationary operands.
  - Projections run data-stationary on the PE: lhsT = X^T chunk [din,128tok],
    rhs = W^T chunk [din, 512 dout] -> PSUM [128 tok, 512 dout]; fp32r dtype
    (full-rate fp32 mode). Bias added via an extra ones-row matmul.
  - K/V token tiles are k-sliced (128 tokens = one neighbor index k0 across
    128 anchors), so projection outputs land directly in the attention-friendly
    layout Kt/Vt [anchor_p, k, h, d].
  - Attention on DVE/ACT: scores = reduce_d(Kt * Q_bcast) (scaled Wq on host),
    exp on ACT, sum/reciprocal, AV = reduce_k(Vt * w_bcast), final scale.
  - O-projection: PE-transpose attn -> data-stationary matmul -> DMA out.
"""

import numpy as np
from contextlib import ExitStack

import concourse.bass as bass
import concourse.tile as tile
from concourse import bacc, mybir
from concourse.bass_utils import run_bass_kernel_spmd
from concourse.masks import make_identity

# ---- problem constants ----
B = 2
V = 2048
D = 512
K = 32
H = 8
Dh = 64
NCORES = 8
VS = V // NCORES          # anchors per core
VT = 128                  # anchors per attention tile
N_VT = VS // VT           # vtiles per batch entry per core
DT = mybir.dt.float32
MMDT = mybir.dt.float32r  # matmul compute dtype (full-rate fp32 mode)
KQ = 4                    # k's per streaming chunk
N_KQ = K // KQ

_PROGRAM_CACHE = {}


RDT = mybir.dt.float32r  # dtype for matmul operand tiles (PE rounds on ingest)


def build_program():
    nc = bacc.Bacc("TRN2", target_bir_lowering=False, debug=False,
                   num_devices=NCORES)

    anchor = nc.dram_tensor("anchor", [B, VS, D], DT, kind="ExternalInput").ap()
    neigh = nc.dram_tensor("neigh", [B, VS, K, D], DT, kind="ExternalInput").ap()
    wqT = nc.dram_tensor("wqT", [D, D], RDT, kind="ExternalInput").ap()
    wkT = nc.dram_tensor("wkT", [D, D], RDT, kind="ExternalInput").ap()
    wvT = nc.dram_tensor("wvT", [D, D], RDT, kind="ExternalInput").ap()
    woT = nc.dram_tensor("woT", [D, D], RDT, kind="ExternalInput").ap()
    biases = nc.dram_tensor("biases", [4, D], RDT, kind="ExternalInput").ap()
    ones_d = nc.dram_tensor("ones", [1, 128], RDT, kind="ExternalInput").ap()
    out = nc.dram_tensor("out", [B, VS, D], DT, kind="ExternalOutput").ap()

    with tile.TileContext(nc) as tc, ExitStack() as ctx:
        const_pool = ctx.enter_context(tc.tile_pool(name="const", bufs=1))
        w_pool = ctx.enter_context(tc.tile_pool(name="weights", bufs=1))
        xnat_pool = ctx.enter_context(tc.tile_pool(name="xnat", bufs=2))
        xT_pool = ctx.enter_context(tc.tile_pool(name="xT", bufs=3))
        kt_pool = ctx.enter_context(tc.tile_pool(name="kt", bufs=2))
        vt_pool = ctx.enter_context(tc.tile_pool(name="vt", bufs=1))
        q_pool = ctx.enter_context(tc.tile_pool(name="q", bufs=2))
        sc_pool = ctx.enter_context(tc.tile_pool(name="scores", bufs=1))
        prod_pool = ctx.enter_context(tc.tile_pool(name="prod", bufs=2))
        attn_pool = ctx.enter_context(tc.tile_pool(name="attn", bufs=1))
        y_pool = ctx.enter_context(tc.tile_pool(name="y", bufs=2))
        tps_pool = ctx.enter_context(
            tc.tile_pool(name="tpsum", bufs=2, space="PSUM"))
        mm_pool = ctx.enter_context(
            tc.tile_pool(name="mmpsum", bufs=2, space="PSUM"))

        # constants
        ident = const_pool.tile([128, 128], DT)
        make_identity(nc, ident[:])
        ones = const_pool.tile([1, 128], RDT)
        nc.sync.dma_start(ones[:], ones_d[:, :])

        # weights: [din(4x128 partition chunks), dout 512]
        wq_sb = w_pool.tile([128, 4, D], RDT)
        wk_sb = w_pool.tile([128, 4, D], RDT)
        wv_sb = w_pool.tile([128, 4, D], RDT)
        wo_sb = w_pool.tile([128, 4, D], RDT)
        for sb, dram in ((wq_sb, wqT), (wk_sb, wkT), (wv_sb, wvT), (wo_sb, woT)):
            for c in range(4):
                nc.sync.dma_start(sb[:, c], dram[c * 128:(c + 1) * 128, :])
        bias_sb = w_pool.tile([1, 4, D], RDT)
        nc.sync.dma_start(bias_sb[:, :, :], biases[:, :].unsqueeze(0))
        bq_sb, bk_sb, bv_sb, bo_sb = (bias_sb[:, i] for i in range(4))

        def transpose_128x512(src_view, dst_tile):
            """src [128, 512] -> dst SBUF [128, 4, 128] ([din_chunk, token])."""
            ps = tps_pool.tile([128, 4, 128], DT, tag="tps")
            for c in range(4):
                nc.tensor.transpose(ps[:, c], src_view[:, c * 128:(c + 1) * 128],
                                    ident[:])
            nc.scalar.copy(dst_tile[:], ps[:])

        def project(xT, w_sb, b_sb, ps):
            """PSUM [128tok, 512] = xT.T @ W^T + ones.T @ bias."""
            for c in range(4):
                nc.tensor.matmul(ps[:], xT[:, c], w_sb[:, c],
                                 start=(c == 0), stop=False)
            nc.tensor.matmul(ps[:], ones[:1, :], b_sb[:1, :],
                             start=False, stop=True)

        for b in range(B):
            for vt in range(N_VT):
                v0 = vt * VT
                # ---- Q projection ----
                q_nat = xnat_pool.tile([128, D], DT, tag="xnat")
                nc.sync.dma_start(q_nat[:], anchor[b, v0:v0 + VT, :])
                qT = xT_pool.tile([128, 4, 128], RDT, tag="xT")
                transpose_128x512(q_nat[:], qT)
                q_ps = mm_pool.tile([128, D], DT, tag="qps", bufs=1)
                project(qT, wq_sb, bq_sb, q_ps)
                q_sb = q_pool.tile([128, H, Dh], DT)
                nc.scalar.copy(q_sb[:], q_ps[:])

                # ---- K/V projections (k-sliced token tiles) + scores ----
                vt_sb = vt_pool.tile([128, K, H, Dh], DT)
                scores = sc_pool.tile([128, H, K], DT, tag="scores")
                for kq in range(N_KQ):
                    ktq = kt_pool.tile([128, KQ, H, Dh], DT)
                    x4 = xnat_pool.tile([128, KQ, D], DT, tag="xnat")
                    k0 = kq * KQ
                    nc.sync.dma_start(
                        x4[:], neigh[b, v0:v0 + VT, k0:k0 + KQ, :])
                    for jj in range(KQ):
                        xT = xT_pool.tile([128, 4, 128], RDT, tag="xT")
                        transpose_128x512(x4[:, jj], xT)
                        k_ps = mm_pool.tile([128, D], DT, tag="kps")
                        project(xT, wk_sb, bk_sb, k_ps)
                        v_ps = mm_pool.tile([128, D], DT, tag="vps")
                        project(xT, wv_sb, bv_sb, v_ps)
                        nc.scalar.copy(ktq[:, jj], k_ps[:])
                        nc.scalar.copy(vt_sb[:, k0 + jj], v_ps[:])
                    # scores for this quarter: prod[kj, h, d] = Kt * Q
                    prod = prod_pool.tile([128, KQ, H, Dh], DT, tag="prod")
                    q_b = q_sb[:].unsqueeze(1).broadcast_to([128, KQ, H, Dh])
                    nc.vector.tensor_tensor(
                        out=prod[:], in0=ktq[:], in1=q_b,
                        op=mybir.AluOpType.mult)
                    sc_slice = scores[:, :, kq * KQ:(kq + 1) * KQ]
                    nc.vector.tensor_reduce(
                        out=sc_slice.transpose([0, 2, 1]), in_=prod[:],
                        axis=mybir.AxisListType.X, op=mybir.AluOpType.add)

                # ---- softmax (exp without max-sub; |scores| <~ 6) ----
                wts = sc_pool.tile([128, H, K], DT, tag="wts")
                nc.scalar.activation(wts[:], scores[:],
                                     mybir.ActivationFunctionType.Exp)
                ssum = sc_pool.tile([128, H], DT, tag="ssum")
                nc.vector.tensor_reduce(
                    out=ssum[:], in_=wts[:], axis=mybir.AxisListType.X,
                    op=mybir.AluOpType.add)
                rec = sc_pool.tile([128, H], DT, tag="rec")
                nc.vector.reciprocal(rec[:], ssum[:])
                rec_b = rec[:].unsqueeze(2).broadcast_to([128, H, K])
                wtsn = sc_pool.tile([128, H, K], DT, tag="wtsn")
                nc.vector.tensor_tensor(out=wtsn[:], in0=wts[:], in1=rec_b,
                                        op=mybir.AluOpType.mult)

                # ---- AV: attn[h, d] = sum_k wts[h,k] * Vt[k,h,d] ----
                acc = None
                for kq in range(N_KQ):
                    prod2 = prod_pool.tile([128, H, Dh, KQ], DT, tag="prod")
                    v_view = vt_sb[:, kq * KQ:(kq + 1) * KQ].transpose(
                        [0, 2, 3, 1])
                    w_view = wtsn[:, :, kq * KQ:(kq + 1) * KQ].unsqueeze(
                        2).broadcast_to([128, H, Dh, KQ])
                    nc.vector.tensor_tensor(
                        out=prod2[:], in0=v_view, in1=w_view,
                        op=mybir.AluOpType.mult)
                    part = attn_pool.tile([128, H, Dh], DT, tag="avp", bufs=2)
                    nc.vector.tensor_reduce(
                        out=part[:], in_=prod2[:], axis=mybir.AxisListType.X,
                        op=mybir.AluOpType.add)
                    if acc is None:
                        acc = part
                    else:
                        nxt = attn_pool.tile([128, H, Dh], DT, tag="avacc",
                                             bufs=2)
                        nc.vector.tensor_add(nxt[:], acc[:], part[:])
                        acc = nxt
                attn = acc

                # ---- O projection ----
                attn_flat = attn[:].rearrange("p h d -> p (h d)")
                aT = xT_pool.tile([128, 4, 128], RDT, tag="xT")
                transpose_128x512(attn_flat, aT)
                y_ps = mm_pool.tile([128, D], DT, tag="yps", bufs=1)
                project(aT, wo_sb, bo_sb, y_ps)
                y_sb = y_pool.tile([128, D], DT)
                nc.scalar.copy(y_sb[:], y_ps[:])
                nc.sync.dma_start(out[b, v0:v0 + VT, :], y_sb[:])

    nc.compile()
    return nc


def get_program():
    if "nc" not in _PROGRAM_CACHE:
        _PROGRAM_CACHE["nc"] = build_program()
    return _PROGRAM_CACHE["nc"]


def make_in_maps(anchor_tokens, neighbor_tokens, Wq, bq, Wk, bk, Wv, bv, Wo, bo):
    scale = np.float32(1.0 / np.sqrt(Dh))
    wqT = np.ascontiguousarray(Wq.T * scale, dtype=np.float32)
    wkT = np.ascontiguousarray(Wk.T, dtype=np.float32)
    wvT = np.ascontiguousarray(Wv.T, dtype=np.float32)
    woT = np.ascontiguousarray(Wo.T, dtype=np.float32)
    biases = np.stack([bq * scale, bk, bv, bo]).astype(np.float32)
    anchor_tokens = np.asarray(anchor_tokens, dtype=np.float32)
    neighbor_tokens = np.asarray(neighbor_tokens, dtype=np.float32)
    in_maps = []
    for c in range(NCORES):
        sl = slice(c * VS, (c + 1) * VS)
        in_maps.append({
            "anchor": np.ascontiguousarray(anchor_tokens[:, sl]),
            "neigh": np.ascontiguousarray(neighbor_tokens[:, sl]),
            "wqT": wqT, "wkT": wkT, "wvT": wvT, "woT": woT,
            "biases": biases, "ones": np.ones((1, 128), np.float32),
        })
    return in_maps


def kernel(**inputs):
    nc = get_program()
    in_maps = make_in_maps(**inputs)
    res = run_bass_kernel_spmd(nc, in_maps, list(range(NCORES)))
    out = np.concatenate([res.results[c]["out"] for c in range(NCORES)],
                         axis=1)
    return out


# revision 6
# speedup vs baseline: 7190.2123x; 7190.2123x over previous
"""Trainium2 Bass kernel for per-anchor local cross-attention.

Problem shapes (hardcoded per contract):
  anchor_tokens   [B=2, V=2048, D=512]
  neighbor_tokens [B=2, V=2048, K=32, D=512]
  Wq/Wk/Wv/Wo     [512, 512], bq/bk/bv/bo [512]
  out             [B, V, D] = attention(anchor over its K neighbors) @ Wo.T + bo

Sharding: V split across 8 cores (attention is local per anchor; weights
replicated). Each core handles Vs = 256 anchors for both batch entries.

Per-core plan (all on one NeuronCore, SPMD over 8):
  - Load anchor / neighbor token tiles naturally ([token, din]); PE-transpose
    128x128 blocks to get [din, token] stationary operands.
  - Projections run data-stationary on the PE: lhsT = X^T chunk [din,128tok],
    rhs = W^T chunk [din, 512 dout] -> PSUM [128 tok, 512 dout]; fp32r dtype
    (full-rate fp32 mode). Bias added via an extra ones-row matmul.
  - K/V token tiles are k-sliced (128 tokens = one neighbor index k0 across
    128 anchors), so projection outputs land directly in the attention-friendly
    layout Kt/Vt [anchor_p, k, h, d].
  - Attention on DVE/ACT: scores = reduce_d(Kt * Q_bcast) (scaled Wq on host),
    exp on ACT, sum/reciprocal, AV = reduce_k(Vt * w_bcast), final scale.
  - O-projection: PE-transpose attn -> data-stationary matmul -> DMA out.
"""

import numpy as np
from contextlib import ExitStack

import concourse.bass as bass
import concourse.tile as tile
from concourse import bacc, mybir
from concourse.bass_utils import run_bass_kernel_spmd
from concourse.masks import make_identity

# ---- problem constants ----
B = 2
V = 2048
D = 512
K = 32
H = 8
Dh = 64
NCORES = 8
VS = V // NCORES          # anchors per core
VT = 128                  # anchors per attention tile
N_VT = VS // VT           # vtiles per batch entry per core
DT = mybir.dt.float32
MMDT = mybir.dt.float32r  # matmul compute dtype (full-rate fp32 mode)
KQ = 4                    # k's per streaming chunk
N_KQ = K // KQ

_PROGRAM_CACHE = {}


RDT = mybir.dt.float32r  # dtype for matmul operand tiles (PE rounds on ingest)


def build_program_reps(reps=1):
    nc = bacc.Bacc("TRN2", target_bir_lowering=False, debug=False,
                   num_devices=NCORES)

    anchor = nc.dram_tensor("anchor", [B, VS, D], DT, kind="ExternalInput").ap()
    neigh = nc.dram_tensor("neigh", [B, VS, K, D], DT, kind="ExternalInput").ap()
    wqT = nc.dram_tensor("wqT", [D, D], RDT, kind="ExternalInput").ap()
    wkT = nc.dram_tensor("wkT", [D, D], RDT, kind="ExternalInput").ap()
    wvT = nc.dram_tensor("wvT", [D, D], RDT, kind="ExternalInput").ap()
    woT = nc.dram_tensor("woT", [D, D], RDT, kind="ExternalInput").ap()
    biases = nc.dram_tensor("biases", [4, D], RDT, kind="ExternalInput").ap()
    ones_d = nc.dram_tensor("ones", [1, 128], RDT, kind="ExternalInput").ap()
    out = nc.dram_tensor("out", [B, VS, D], DT, kind="ExternalOutput").ap()

    with tile.TileContext(nc) as tc, ExitStack() as ctx:
        const_pool = ctx.enter_context(tc.tile_pool(name="const", bufs=1))
        w_pool = ctx.enter_context(tc.tile_pool(name="weights", bufs=1))
        xnat_pool = ctx.enter_context(tc.tile_pool(name="xnat", bufs=2))
        xT_pool = ctx.enter_context(tc.tile_pool(name="xT", bufs=3))
        kt_pool = ctx.enter_context(tc.tile_pool(name="kt", bufs=2))
        vt_pool = ctx.enter_context(tc.tile_pool(name="vt", bufs=1))
        q_pool = ctx.enter_context(tc.tile_pool(name="q", bufs=2))
        sc_pool = ctx.enter_context(tc.tile_pool(name="scores", bufs=1))
        prod_pool = ctx.enter_context(tc.tile_pool(name="prod", bufs=2))
        attn_pool = ctx.enter_context(tc.tile_pool(name="attn", bufs=1))
        y_pool = ctx.enter_context(tc.tile_pool(name="y", bufs=2))
        tps_pool = ctx.enter_context(
            tc.tile_pool(name="tpsum", bufs=2, space="PSUM"))
        mm_pool = ctx.enter_context(
            tc.tile_pool(name="mmpsum", bufs=2, space="PSUM"))

        # constants
        ident = const_pool.tile([128, 128], DT)
        make_identity(nc, ident[:])
        ones = const_pool.tile([1, 128], RDT)
        nc.sync.dma_start(ones[:], ones_d[:, :])

        # weights: [din(4x128 partition chunks), dout 512]
        wq_sb = w_pool.tile([128, 4, D], RDT)
        wk_sb = w_pool.tile([128, 4, D], RDT)
        wv_sb = w_pool.tile([128, 4, D], RDT)
        wo_sb = w_pool.tile([128, 4, D], RDT)
        for sb, dram in ((wq_sb, wqT), (wk_sb, wkT), (wv_sb, wvT), (wo_sb, woT)):
            for c in range(4):
                nc.sync.dma_start(sb[:, c], dram[c * 128:(c + 1) * 128, :])
        bias_sb = w_pool.tile([1, 4, D], RDT)
        nc.sync.dma_start(bias_sb[:, :, :], biases[:, :].unsqueeze(0))
        bq_sb, bk_sb, bv_sb, bo_sb = (bias_sb[:, i] for i in range(4))

        def transpose_128x512(src_view, dst_tile):
            """src [128, 512] -> dst SBUF [128, 4, 128] ([din_chunk, token])."""
            ps = tps_pool.tile([128, 4, 128], DT, tag="tps")
            for c in range(4):
                nc.tensor.transpose(ps[:, c], src_view[:, c * 128:(c + 1) * 128],
                                    ident[:])
            nc.scalar.copy(dst_tile[:], ps[:])

        def project(xT, w_sb, b_sb, ps):
            """PSUM [128tok, 512] = xT.T @ W^T + ones.T @ bias."""
            for c in range(4):
                nc.tensor.matmul(ps[:], xT[:, c], w_sb[:, c],
                                 start=(c == 0), stop=False)
            nc.tensor.matmul(ps[:], ones[:1, :], b_sb[:1, :],
                             start=False, stop=True)

        for _rep in range(reps):
          for b in range(B):
            for vt in range(N_VT):
                v0 = vt * VT
                # ---- Q projection ----
                q_nat = xnat_pool.tile([128, D], DT, tag="xnat")
                nc.sync.dma_start(q_nat[:], anchor[b, v0:v0 + VT, :])
                qT = xT_pool.tile([128, 4, 128], RDT, tag="xT")
                transpose_128x512(q_nat[:], qT)
                q_ps = mm_pool.tile([128, D], DT, tag="qps", bufs=1)
                project(qT, wq_sb, bq_sb, q_ps)
                q_sb = q_pool.tile([128, H, Dh], DT)
                nc.scalar.copy(q_sb[:], q_ps[:])

                # ---- K/V projections (k-sliced token tiles) + scores ----
                vt_sb = vt_pool.tile([128, K, H, Dh], DT)
                scores = sc_pool.tile([128, H, K], DT, tag="scores")
                for kq in range(N_KQ):
                    ktq = kt_pool.tile([128, KQ, H, Dh], DT)
                    x4 = xnat_pool.tile([128, KQ, D], DT, tag="xnat")
                    k0 = kq * KQ
                    nc.sync.dma_start(
                        x4[:], neigh[b, v0:v0 + VT, k0:k0 + KQ, :])
                    for jj in range(KQ):
                        xT = xT_pool.tile([128, 4, 128], RDT, tag="xT")
                        transpose_128x512(x4[:, jj], xT)
                        k_ps = mm_pool.tile([128, D], DT, tag="kps")
                        project(xT, wk_sb, bk_sb, k_ps)
                        v_ps = mm_pool.tile([128, D], DT, tag="vps")
                        project(xT, wv_sb, bv_sb, v_ps)
                        nc.scalar.copy(ktq[:, jj], k_ps[:])
                        nc.scalar.copy(vt_sb[:, k0 + jj], v_ps[:])
                    # scores for this quarter: prod[kj, h, d] = Kt * Q
                    prod = prod_pool.tile([128, KQ, H, Dh], DT, tag="prod")
                    q_b = q_sb[:].unsqueeze(1).broadcast_to([128, KQ, H, Dh])
                    nc.vector.tensor_tensor(
                        out=prod[:], in0=ktq[:], in1=q_b,
                        op=mybir.AluOpType.mult)
                    sc_slice = scores[:, :, kq * KQ:(kq + 1) * KQ]
                    nc.vector.tensor_reduce(
                        out=sc_slice.transpose([0, 2, 1]), in_=prod[:],
                        axis=mybir.AxisListType.X, op=mybir.AluOpType.add)

                # ---- softmax (exp without max-sub; |scores| <~ 6) ----
                wts = sc_pool.tile([128, H, K], DT, tag="wts")
                nc.scalar.activation(wts[:], scores[:],
                                     mybir.ActivationFunctionType.Exp)
                ssum = sc_pool.tile([128, H], DT, tag="ssum")
                nc.vector.tensor_reduce(
                    out=ssum[:], in_=wts[:], axis=mybir.AxisListType.X,
                    op=mybir.AluOpType.add)
                rec = sc_pool.tile([128, H], DT, tag="rec")
                nc.vector.reciprocal(rec[:], ssum[:])
                rec_b = rec[:].unsqueeze(2).broadcast_to([128, H, K])
                wtsn = sc_pool.tile([128, H, K], DT, tag="wtsn")
                nc.vector.tensor_tensor(out=wtsn[:], in0=wts[:], in1=rec_b,
                                        op=mybir.AluOpType.mult)

                # ---- AV: attn[h, d] = sum_k wts[h,k] * Vt[k,h,d] ----
                acc = None
                for kq in range(N_KQ):
                    prod2 = prod_pool.tile([128, H, Dh, KQ], DT, tag="prod")
                    v_view = vt_sb[:, kq * KQ:(kq + 1) * KQ].transpose(
                        [0, 2, 3, 1])
                    w_view = wtsn[:, :, kq * KQ:(kq + 1) * KQ].unsqueeze(
                        2).broadcast_to([128, H, Dh, KQ])
                    nc.vector.tensor_tensor(
                        out=prod2[:], in0=v_view, in1=w_view,
                        op=mybir.AluOpType.mult)
                    part = attn_pool.tile([128, H, Dh], DT, tag="avp", bufs=2)
                    nc.vector.tensor_reduce(
                        out=part[:], in_=prod2[:], axis=mybir.AxisListType.X,
                        op=mybir.AluOpType.add)
                    if acc is None:
                        acc = part
                    else:
                        nxt = attn_pool.tile([128, H, Dh], DT, tag="avacc",
                                             bufs=2)
                        nc.vector.tensor_add(nxt[:], acc[:], part[:])
                        acc = nxt
                attn = acc

                # ---- O projection ----
                attn_flat = attn[:].rearrange("p h d -> p (h d)")
                aT = xT_pool.tile([128, 4, 128], RDT, tag="xT")
                transpose_128x512(attn_flat, aT)
                y_ps = mm_pool.tile([128, D], DT, tag="yps", bufs=1)
                project(aT, wo_sb, bo_sb, y_ps)
                y_sb = y_pool.tile([128, D], DT)
                nc.scalar.copy(y_sb[:], y_ps[:])
                nc.sync.dma_start(out[b, v0:v0 + VT, :], y_sb[:])

    nc.compile()
    return nc


def build_program():
    return build_program_reps(1)


def get_program():
    if "nc" not in _PROGRAM_CACHE:
        _PROGRAM_CACHE["nc"] = build_program()
    return _PROGRAM_CACHE["nc"]


def make_in_maps(anchor_tokens, neighbor_tokens, Wq, bq, Wk, bk, Wv, bv, Wo, bo):
    scale = np.float32(1.0 / np.sqrt(Dh))
    wqT = np.ascontiguousarray(Wq.T * scale, dtype=np.float32)
    wkT = np.ascontiguousarray(Wk.T, dtype=np.float32)
    wvT = np.ascontiguousarray(Wv.T, dtype=np.float32)
    woT = np.ascontiguousarray(Wo.T, dtype=np.float32)
    biases = np.stack([bq * scale, bk, bv, bo]).astype(np.float32)
    anchor_tokens = np.asarray(anchor_tokens, dtype=np.float32)
    neighbor_tokens = np.asarray(neighbor_tokens, dtype=np.float32)
    in_maps = []
    for c in range(NCORES):
        sl = slice(c * VS, (c + 1) * VS)
        in_maps.append({
            "anchor": np.ascontiguousarray(anchor_tokens[:, sl]),
            "neigh": np.ascontiguousarray(neighbor_tokens[:, sl]),
            "wqT": wqT, "wkT": wkT, "wvT": wvT, "woT": woT,
            "biases": biases, "ones": np.ones((1, 128), np.float32),
        })
    return in_maps


def kernel(**inputs):
    nc = get_program()
    in_maps = make_in_maps(**inputs)
    res = run_bass_kernel_spmd(nc, in_maps, list(range(NCORES)))
    out = np.concatenate([res.results[c]["out"] for c in range(NCORES)],
                         axis=1)
    return out


# revision 8
# speedup vs baseline: 13968.7061x; 1.9427x over previous
"""Trainium2 Bass kernel for per-anchor local cross-attention.

Problem shapes (hardcoded per contract):
  anchor_tokens   [B=2, V=2048, D=512]
  neighbor_tokens [B=2, V=2048, K=32, D=512]
  Wq/Wk/Wv/Wo     [512, 512], bq/bk/bv/bo [512]
  out             [B, V, D] = attention(anchor over its K neighbors) @ Wo.T + bo

Sharding: V split across 8 cores (attention is local per anchor; weights
replicated). Each core handles Vs = 256 anchors for both batch entries.

Per-core plan (all on one NeuronCore, SPMD over 8):
  - Load anchor / neighbor token tiles naturally ([token, din]); PE-transpose
    128x128 blocks to get [din, token] stationary operands.
  - Projections run data-stationary on the PE: lhsT = X^T chunk [din,128tok],
    rhs = W^T chunk [din, 512 dout] -> PSUM [128 tok, 512 dout]; fp32r dtype
    (full-rate fp32 mode). Bias added via an extra ones-row matmul.
  - K/V token tiles are k-sliced (128 tokens = one neighbor index k0 across
    128 anchors), so projection outputs land directly in the attention-friendly
    layout Kt/Vt [anchor_p, k, h, d].
  - Attention on DVE/ACT: scores = reduce_d(Kt * Q_bcast) (scaled Wq on host),
    exp on ACT, sum/reciprocal, AV = reduce_k(Vt * w_bcast), final scale.
  - O-projection: PE-transpose attn -> data-stationary matmul -> DMA out.
"""

import numpy as np
from contextlib import ExitStack

import concourse.bass as bass
import concourse.tile as tile
from concourse import bacc, mybir
from concourse.bass_utils import run_bass_kernel_spmd
from concourse.masks import make_identity

# ---- problem constants ----
B = 2
V = 2048
D = 512
K = 32
H = 8
Dh = 64
NCORES = 8
VS = V // NCORES          # anchors per core
VT = 128                  # anchors per attention tile
N_VT = VS // VT           # vtiles per batch entry per core
DT = mybir.dt.float32
MMDT = mybir.dt.float32r  # matmul compute dtype (full-rate fp32 mode)
KQ = 4                    # k's per streaming chunk
N_KQ = K // KQ

_PROGRAM_CACHE = {}


RDT = mybir.dt.float32r  # dtype for matmul operand tiles (PE rounds on ingest)


def build_program_reps(reps=1):
    nc = bacc.Bacc("TRN2", target_bir_lowering=False, debug=False,
                   num_devices=NCORES)

    anchor = nc.dram_tensor("anchor", [B, VS, D], DT, kind="ExternalInput").ap()
    neigh = nc.dram_tensor("neigh", [B, VS, K, D], DT, kind="ExternalInput").ap()
    wqT = nc.dram_tensor("wqT", [D, D], RDT, kind="ExternalInput").ap()
    wkT = nc.dram_tensor("wkT", [D, D], RDT, kind="ExternalInput").ap()
    wvT = nc.dram_tensor("wvT", [D, D], RDT, kind="ExternalInput").ap()
    woT = nc.dram_tensor("woT", [D, D], RDT, kind="ExternalInput").ap()
    biases = nc.dram_tensor("biases", [4, D], RDT, kind="ExternalInput").ap()
    ones_d = nc.dram_tensor("ones", [1, 128], RDT, kind="ExternalInput").ap()
    out = nc.dram_tensor("out", [B, VS, D], DT, kind="ExternalOutput").ap()

    with tile.TileContext(nc) as tc, ExitStack() as ctx:
        const_pool = ctx.enter_context(tc.tile_pool(name="const", bufs=1))
        w_pool = ctx.enter_context(tc.tile_pool(name="weights", bufs=1))
        xnat_pool = ctx.enter_context(tc.tile_pool(name="xnat", bufs=2))
        xT_pool = ctx.enter_context(tc.tile_pool(name="xT", bufs=3))
        kt_pool = ctx.enter_context(tc.tile_pool(name="kt", bufs=2))
        vt_pool = ctx.enter_context(tc.tile_pool(name="vt", bufs=1))
        q_pool = ctx.enter_context(tc.tile_pool(name="q", bufs=2))
        sc_pool = ctx.enter_context(tc.tile_pool(name="scores", bufs=2))
        prod_pool = ctx.enter_context(tc.tile_pool(name="prod", bufs=2))
        attn_pool = ctx.enter_context(tc.tile_pool(name="attn", bufs=2))
        y_pool = ctx.enter_context(tc.tile_pool(name="y", bufs=2))
        tps_pool = ctx.enter_context(
            tc.tile_pool(name="tpsum", bufs=2, space="PSUM"))
        mm_pool = ctx.enter_context(
            tc.tile_pool(name="mmpsum", bufs=2, space="PSUM"))

        # constants
        ident = const_pool.tile([128, 128], DT)
        make_identity(nc, ident[:])
        ones = const_pool.tile([1, 128], RDT)
        nc.sync.dma_start(ones[:], ones_d[:, :])

        # weights: [din(4x128 partition chunks), dout 512]
        wq_sb = w_pool.tile([128, 4, D], RDT)
        wk_sb = w_pool.tile([128, 4, D], RDT)
        wv_sb = w_pool.tile([128, 4, D], RDT)
        wo_sb = w_pool.tile([128, 4, D], RDT)
        for sb, dram in ((wq_sb, wqT), (wk_sb, wkT), (wv_sb, wvT), (wo_sb, woT)):
            for c in range(4):
                nc.sync.dma_start(sb[:, c], dram[c * 128:(c + 1) * 128, :])
        bias_sb = w_pool.tile([1, 4, D], RDT)
        nc.sync.dma_start(bias_sb[:, :, :], biases[:, :].unsqueeze(0))
        bq_sb, bk_sb, bv_sb, bo_sb = (bias_sb[:, i] for i in range(4))

        def transpose_128x512(src_view, dst_tile):
            """src [128, 512] -> dst SBUF [128, 4, 128] ([din_chunk, token])."""
            ps = tps_pool.tile([128, 4, 128], DT, tag="tps")
            for c in range(4):
                nc.tensor.transpose(ps[:, c], src_view[:, c * 128:(c + 1) * 128],
                                    ident[:])
            nc.scalar.copy(dst_tile[:], ps[:])

        def project(xT, w_sb, b_sb, ps):
            """PSUM [128tok, 512] = xT.T @ W^T + ones.T @ bias."""
            for c in range(4):
                nc.tensor.matmul(ps[:], xT[:, c], w_sb[:, c],
                                 start=(c == 0), stop=False)
            nc.tensor.matmul(ps[:], ones[:1, :], b_sb[:1, :],
                             start=False, stop=True)

        for _rep in range(reps):
          for b in range(B):
            for vt in range(N_VT):
                v0 = vt * VT
                # ---- Q projection ----
                q_nat = xnat_pool.tile([128, D], DT, tag="xnat")
                nc.sync.dma_start(q_nat[:], anchor[b, v0:v0 + VT, :])
                qT = xT_pool.tile([128, 4, 128], RDT, tag="xT")
                transpose_128x512(q_nat[:], qT)
                q_ps = mm_pool.tile([128, D], DT, tag="qps", bufs=1)
                project(qT, wq_sb, bq_sb, q_ps)
                q_sb = q_pool.tile([128, H, Dh], DT)
                nc.scalar.copy(q_sb[:], q_ps[:])

                # ---- K/V projections (k-sliced token tiles) + scores ----
                vt_sb = vt_pool.tile([128, K, H, Dh], DT)
                scores = sc_pool.tile([128, H, K], DT, tag="scores")
                for kq in range(N_KQ):
                    ktq = kt_pool.tile([128, KQ, H, Dh], DT)
                    x4 = xnat_pool.tile([128, KQ, D], DT, tag="xnat")
                    k0 = kq * KQ
                    nc.sync.dma_start(
                        x4[:], neigh[b, v0:v0 + VT, k0:k0 + KQ, :])
                    for jj in range(KQ):
                        xT = xT_pool.tile([128, 4, 128], RDT, tag="xT")
                        transpose_128x512(x4[:, jj], xT)
                        k_ps = mm_pool.tile([128, D], DT, tag="kps")
                        project(xT, wk_sb, bk_sb, k_ps)
                        v_ps = mm_pool.tile([128, D], DT, tag="vps")
                        project(xT, wv_sb, bv_sb, v_ps)
                        nc.scalar.copy(ktq[:, jj], k_ps[:])
                        nc.scalar.copy(vt_sb[:, k0 + jj], v_ps[:])
                    # scores for this quarter: prod[kj, h, d] = Kt * Q
                    prod = prod_pool.tile([128, KQ, H, Dh], DT, tag="prod", bufs=3)
                    q_b = q_sb[:].unsqueeze(1).broadcast_to([128, KQ, H, Dh])
                    nc.vector.tensor_tensor(
                        out=prod[:], in0=ktq[:], in1=q_b,
                        op=mybir.AluOpType.mult)
                    sc_slice = scores[:, :, kq * KQ:(kq + 1) * KQ]
                    nc.vector.tensor_reduce(
                        out=sc_slice.transpose([0, 2, 1]), in_=prod[:],
                        axis=mybir.AxisListType.X, op=mybir.AluOpType.add)

                # ---- softmax (exp without max-sub; |scores| <~ 6) ----
                wts = sc_pool.tile([128, H, K], DT, tag="wts")
                nc.scalar.activation(wts[:], scores[:],
                                     mybir.ActivationFunctionType.Exp)
                ssum = sc_pool.tile([128, H], DT, tag="ssum")
                nc.vector.tensor_reduce(
                    out=ssum[:], in_=wts[:], axis=mybir.AxisListType.X,
                    op=mybir.AluOpType.add)
                rec = sc_pool.tile([128, H], DT, tag="rec")
                nc.vector.reciprocal(rec[:], ssum[:])
                rec_b = rec[:].unsqueeze(2).broadcast_to([128, H, K])
                wtsn = sc_pool.tile([128, H, K], DT, tag="wtsn")
                nc.vector.tensor_tensor(out=wtsn[:], in0=wts[:], in1=rec_b,
                                        op=mybir.AluOpType.mult)

                # ---- AV: attn[h, d] = sum_k wts[h,k] * Vt[k,h,d] ----
                acc = None
                for kq in range(N_KQ):
                    prod2 = prod_pool.tile([128, H, Dh, KQ], DT, tag="prod", bufs=3)
                    v_view = vt_sb[:, kq * KQ:(kq + 1) * KQ].transpose(
                        [0, 2, 3, 1])
                    w_view = wtsn[:, :, kq * KQ:(kq + 1) * KQ].unsqueeze(
                        2).broadcast_to([128, H, Dh, KQ])
                    nc.gpsimd.tensor_tensor(
                        out=prod2[:], in0=v_view, in1=w_view,
                        op=mybir.AluOpType.mult)
                    part = attn_pool.tile([128, H, Dh], DT, tag="avp", bufs=2)
                    nc.vector.tensor_reduce(
                        out=part[:], in_=prod2[:], axis=mybir.AxisListType.X,
                        op=mybir.AluOpType.add)
                    if acc is None:
                        acc = part
                    else:
                        nxt = attn_pool.tile([128, H, Dh], DT, tag="avacc",
                                             bufs=2)
                        nc.vector.tensor_add(nxt[:], acc[:], part[:])
                        acc = nxt
                attn = acc

                # ---- O projection ----
                attn_flat = attn[:].rearrange("p h d -> p (h d)")
                aT = xT_pool.tile([128, 4, 128], RDT, tag="xT")
                transpose_128x512(attn_flat, aT)
                y_ps = mm_pool.tile([128, D], DT, tag="yps", bufs=1)
                project(aT, wo_sb, bo_sb, y_ps)
                y_sb = y_pool.tile([128, D], DT)
                nc.scalar.copy(y_sb[:], y_ps[:])
                nc.sync.dma_start(out[b, v0:v0 + VT, :], y_sb[:])

    nc.compile()
    return nc


def build_program():
    return build_program_reps(1)


def get_program():
    if "nc" not in _PROGRAM_CACHE:
        _PROGRAM_CACHE["nc"] = build_program()
    return _PROGRAM_CACHE["nc"]


def make_in_maps(anchor_tokens, neighbor_tokens, Wq, bq, Wk, bk, Wv, bv, Wo, bo):
    scale = np.float32(1.0 / np.sqrt(Dh))
    wqT = np.ascontiguousarray(Wq.T * scale, dtype=np.float32)
    wkT = np.ascontiguousarray(Wk.T, dtype=np.float32)
    wvT = np.ascontiguousarray(Wv.T, dtype=np.float32)
    woT = np.ascontiguousarray(Wo.T, dtype=np.float32)
    biases = np.stack([bq * scale, bk, bv, bo]).astype(np.float32)
    anchor_tokens = np.asarray(anchor_tokens, dtype=np.float32)
    neighbor_tokens = np.asarray(neighbor_tokens, dtype=np.float32)
    in_maps = []
    for c in range(NCORES):
        sl = slice(c * VS, (c + 1) * VS)
        in_maps.append({
            "anchor": np.ascontiguousarray(anchor_tokens[:, sl]),
            "neigh": np.ascontiguousarray(neighbor_tokens[:, sl]),
            "wqT": wqT, "wkT": wkT, "wvT": wvT, "woT": woT,
            "biases": biases, "ones": np.ones((1, 128), np.float32),
        })
    return in_maps


def kernel(**inputs):
    nc = get_program()
    in_maps = make_in_maps(**inputs)
    res = run_bass_kernel_spmd(nc, in_maps, list(range(NCORES)))
    out = np.concatenate([res.results[c]["out"] for c in range(NCORES)],
                         axis=1)
    return out


# revision 9
# speedup vs baseline: 18135.3020x; 1.2983x over previous
"""Trainium2 Bass kernel for per-anchor local cross-attention.

Problem shapes (hardcoded per contract):
  anchor_tokens   [B=2, V=2048, D=512]
  neighbor_tokens [B=2, V=2048, K=32, D=512]
  Wq/Wk/Wv/Wo     [512, 512], bq/bk/bv/bo [512]
  out             [B, V, D] = attention(anchor over its K neighbors) @ Wo.T + bo

Sharding: V split across 8 cores (attention is local per anchor; weights
replicated). Each core handles Vs = 256 anchors for both batch entries.

Per-core plan (all on one NeuronCore, SPMD over 8):
  - Load anchor / neighbor token tiles naturally ([token, din]); PE-transpose
    128x128 blocks to get [din, token] stationary operands.
  - Projections run data-stationary on the PE: lhsT = X^T chunk [din,128tok],
    rhs = W^T chunk [din, 512 dout] -> PSUM [128 tok, 512 dout]; fp32r dtype
    (full-rate fp32 mode). Bias added via an extra ones-row matmul.
  - K/V token tiles are k-sliced (128 tokens = one neighbor index k0 across
    128 anchors), so projection outputs land directly in the attention-friendly
    layout Kt/Vt [anchor_p, k, h, d].
  - Attention on DVE/ACT: scores = reduce_d(Kt * Q_bcast) (scaled Wq on host),
    exp on ACT, sum/reciprocal, AV = reduce_k(Vt * w_bcast), final scale.
  - O-projection: PE-transpose attn -> data-stationary matmul -> DMA out.
"""

import numpy as np
from contextlib import ExitStack

import concourse.bass as bass
import concourse.tile as tile
from concourse import bacc, mybir
from concourse.bass_utils import run_bass_kernel_spmd
from concourse.masks import make_identity

# ---- problem constants ----
B = 2
V = 2048
D = 512
K = 32
H = 8
Dh = 64
NCORES = 8
VS = V // NCORES          # anchors per core
VT = 128                  # anchors per attention tile
N_VT = VS // VT           # vtiles per batch entry per core
DT = mybir.dt.float32
MMDT = mybir.dt.float32r  # matmul compute dtype (full-rate fp32 mode)
KQ = 4                    # k's per streaming chunk
N_KQ = K // KQ

_PROGRAM_CACHE = {}


RDT = mybir.dt.float32r  # dtype for matmul operand tiles (PE rounds on ingest)


def build_program_reps(reps=1):
    nc = bacc.Bacc("TRN2", target_bir_lowering=False, debug=False,
                   num_devices=NCORES)

    anchor = nc.dram_tensor("anchor", [B, VS, D], DT, kind="ExternalInput").ap()
    neigh = nc.dram_tensor("neigh", [B, VS, K, D], DT, kind="ExternalInput").ap()
    wqT = nc.dram_tensor("wqT", [D, D], RDT, kind="ExternalInput").ap()
    wkT = nc.dram_tensor("wkT", [D, D], RDT, kind="ExternalInput").ap()
    wvT = nc.dram_tensor("wvT", [D, D], RDT, kind="ExternalInput").ap()
    woT = nc.dram_tensor("woT", [D, D], RDT, kind="ExternalInput").ap()
    biases = nc.dram_tensor("biases", [4, D], RDT, kind="ExternalInput").ap()
    ones_d = nc.dram_tensor("ones", [1, 128], RDT, kind="ExternalInput").ap()
    out = nc.dram_tensor("out", [B, VS, D], DT, kind="ExternalOutput").ap()

    with tile.TileContext(nc) as tc, ExitStack() as ctx:
        const_pool = ctx.enter_context(tc.tile_pool(name="const", bufs=1))
        w_pool = ctx.enter_context(tc.tile_pool(name="weights", bufs=1))
        xnat_pool = ctx.enter_context(tc.tile_pool(name="xnat", bufs=2))
        xT_pool = ctx.enter_context(tc.tile_pool(name="xT", bufs=3))
        kt_pool = ctx.enter_context(tc.tile_pool(name="kt", bufs=2))
        q_pool = ctx.enter_context(tc.tile_pool(name="q", bufs=2))
        sc_pool = ctx.enter_context(tc.tile_pool(name="scores", bufs=2))
        prod_pool = ctx.enter_context(tc.tile_pool(name="prod", bufs=2))
        attn_pool = ctx.enter_context(tc.tile_pool(name="attn", bufs=2))
        y_pool = ctx.enter_context(tc.tile_pool(name="y", bufs=2))
        tps_pool = ctx.enter_context(
            tc.tile_pool(name="tpsum", bufs=2, space="PSUM"))
        mm_pool = ctx.enter_context(
            tc.tile_pool(name="mmpsum", bufs=2, space="PSUM"))

        # constants
        ident = const_pool.tile([128, 128], DT)
        make_identity(nc, ident[:])
        ones = const_pool.tile([1, 128], RDT)
        nc.sync.dma_start(ones[:], ones_d[:, :])

        # weights: [din(4x128 partition chunks), dout 512]
        wq_sb = w_pool.tile([128, 4, D], RDT)
        wk_sb = w_pool.tile([128, 4, D], RDT)
        wv_sb = w_pool.tile([128, 4, D], RDT)
        wo_sb = w_pool.tile([128, 4, D], RDT)
        for sb, dram in ((wq_sb, wqT), (wk_sb, wkT), (wv_sb, wvT), (wo_sb, woT)):
            for c in range(4):
                nc.sync.dma_start(sb[:, c], dram[c * 128:(c + 1) * 128, :])
        bias_sb = w_pool.tile([1, 4, D], RDT)
        nc.sync.dma_start(bias_sb[:, :, :], biases[:, :].unsqueeze(0))
        bq_sb, bk_sb, bv_sb, bo_sb = (bias_sb[:, i] for i in range(4))

        def transpose_128x512(src_view, dst_tile):
            """src [128, 512] -> dst SBUF [128, 4, 128] ([din_chunk, token])."""
            ps = tps_pool.tile([128, 4, 128], DT, tag="tps")
            for c in range(4):
                nc.tensor.transpose(ps[:, c], src_view[:, c * 128:(c + 1) * 128],
                                    ident[:])
            nc.scalar.copy(dst_tile[:], ps[:])

        def project(xT, w_sb, b_sb, ps):
            """PSUM [128tok, 512] = xT.T @ W^T + ones.T @ bias."""
            for c in range(4):
                nc.tensor.matmul(ps[:], xT[:, c], w_sb[:, c],
                                 start=(c == 0), stop=False)
            nc.tensor.matmul(ps[:], ones[:1, :], b_sb[:1, :],
                             start=False, stop=True)

        for _rep in range(reps):
          for b in range(B):
            for vt in range(N_VT):
                v0 = vt * VT
                # ---- Q projection ----
                q_nat = xnat_pool.tile([128, D], DT, tag="xnat")
                nc.sync.dma_start(q_nat[:], anchor[b, v0:v0 + VT, :])
                qT = xT_pool.tile([128, 4, 128], RDT, tag="xT")
                transpose_128x512(q_nat[:], qT)
                q_ps = mm_pool.tile([128, D], DT, tag="qps", bufs=1)
                project(qT, wq_sb, bq_sb, q_ps)
                q_sb = q_pool.tile([128, H, Dh], DT)
                nc.scalar.copy(q_sb[:], q_ps[:])

                # ---- streamed K/V projections + online-softmax attention ----
                ssum_acc = None
                av_acc = None
                for kq in range(N_KQ):
                    ktq = kt_pool.tile([128, KQ, H, Dh], DT, tag="ktq", bufs=3)
                    vtq = kt_pool.tile([128, KQ, H, Dh], DT, tag="vtq", bufs=3)
                    x4 = xnat_pool.tile([128, KQ, D], DT, tag="xnat")
                    k0 = kq * KQ
                    nc.sync.dma_start(
                        x4[:], neigh[b, v0:v0 + VT, k0:k0 + KQ, :])
                    for jj in range(KQ):
                        xT = xT_pool.tile([128, 4, 128], RDT, tag="xT")
                        transpose_128x512(x4[:, jj], xT)
                        k_ps = mm_pool.tile([128, D], DT, tag="kps")
                        project(xT, wk_sb, bk_sb, k_ps)
                        v_ps = mm_pool.tile([128, D], DT, tag="vps")
                        project(xT, wv_sb, bv_sb, v_ps)
                        nc.scalar.copy(ktq[:, jj], k_ps[:])
                        nc.scalar.copy(vtq[:, jj], v_ps[:])
                    # scores chunk: prod[kj, h, d] = Kt * Q, reduce over d
                    prod = prod_pool.tile([128, KQ, H, Dh], DT, tag="prod",
                                          bufs=3)
                    q_b = q_sb[:].unsqueeze(1).broadcast_to([128, KQ, H, Dh])
                    nc.vector.tensor_tensor(
                        out=prod[:], in0=ktq[:], in1=q_b,
                        op=mybir.AluOpType.mult)
                    scq = sc_pool.tile([128, H, KQ], DT, tag="scq", bufs=2)
                    nc.vector.tensor_reduce(
                        out=scq[:].transpose([0, 2, 1]), in_=prod[:],
                        axis=mybir.AxisListType.X, op=mybir.AluOpType.add)
                    # exp (no max-sub: |scores| <~ 6 is fp32-safe)
                    wq_t = sc_pool.tile([128, H, KQ], DT, tag="wq", bufs=2)
                    nc.scalar.activation(wq_t[:], scq[:],
                                         mybir.ActivationFunctionType.Exp)
                    sp = sc_pool.tile([128, H], DT, tag="sp", bufs=2)
                    nc.vector.tensor_reduce(
                        out=sp[:], in_=wq_t[:], axis=mybir.AxisListType.X,
                        op=mybir.AluOpType.add)
                    if ssum_acc is None:
                        ssum_acc = sp
                    else:
                        nsa = sc_pool.tile([128, H], DT, tag="ssacc", bufs=2)
                        nc.vector.tensor_add(nsa[:], ssum_acc[:], sp[:])
                        ssum_acc = nsa
                    # AV partial: prod2[h, d, kj] = V * w (GPSIMD), reduce kj
                    prod2 = prod_pool.tile([128, H, Dh, KQ], DT, tag="prod",
                                           bufs=3)
                    v_view = vtq[:].transpose([0, 2, 3, 1])
                    w_view = wq_t[:].unsqueeze(2).broadcast_to(
                        [128, H, Dh, KQ])
                    nc.gpsimd.tensor_tensor(
                        out=prod2[:], in0=v_view, in1=w_view,
                        op=mybir.AluOpType.mult)
                    part = attn_pool.tile([128, H, Dh], DT, tag="avp", bufs=2)
                    nc.vector.tensor_reduce(
                        out=part[:], in_=prod2[:], axis=mybir.AxisListType.X,
                        op=mybir.AluOpType.add)
                    if av_acc is None:
                        av_acc = part
                    else:
                        nxt = attn_pool.tile([128, H, Dh], DT, tag="avacc",
                                             bufs=2)
                        nc.vector.tensor_add(nxt[:], av_acc[:], part[:])
                        av_acc = nxt
                # normalize
                rec = sc_pool.tile([128, H], DT, tag="rec", bufs=2)
                nc.vector.reciprocal(rec[:], ssum_acc[:])
                attn = attn_pool.tile([128, H, Dh], DT, tag="attn", bufs=2)
                rec_b = rec[:].unsqueeze(2).broadcast_to([128, H, Dh])
                nc.vector.tensor_tensor(out=attn[:], in0=av_acc[:], in1=rec_b,
                                        op=mybir.AluOpType.mult)

                # ---- O projection ----
                attn_flat = attn[:].rearrange("p h d -> p (h d)")
                aT = xT_pool.tile([128, 4, 128], RDT, tag="xT")
                transpose_128x512(attn_flat, aT)
                y_ps = mm_pool.tile([128, D], DT, tag="yps", bufs=1)
                project(aT, wo_sb, bo_sb, y_ps)
                y_sb = y_pool.tile([128, D], DT)
                nc.scalar.copy(y_sb[:], y_ps[:])
                nc.sync.dma_start(out[b, v0:v0 + VT, :], y_sb[:])

    nc.compile()
    return nc


def build_program():
    return build_program_reps(1)


def get_program():
    if "nc" not in _PROGRAM_CACHE:
        _PROGRAM_CACHE["nc"] = build_program()
    return _PROGRAM_CACHE["nc"]


def make_in_maps(anchor_tokens, neighbor_tokens, Wq, bq, Wk, bk, Wv, bv, Wo, bo):
    scale = np.float32(1.0 / np.sqrt(Dh))
    wqT = np.ascontiguousarray(Wq.T * scale, dtype=np.float32)
    wkT = np.ascontiguousarray(Wk.T, dtype=np.float32)
    wvT = np.ascontiguousarray(Wv.T, dtype=np.float32)
    woT = np.ascontiguousarray(Wo.T, dtype=np.float32)
    biases = np.stack([bq * scale, bk, bv, bo]).astype(np.float32)
    anchor_tokens = np.asarray(anchor_tokens, dtype=np.float32)
    neighbor_tokens = np.asarray(neighbor_tokens, dtype=np.float32)
    in_maps = []
    for c in range(NCORES):
        sl = slice(c * VS, (c + 1) * VS)
        in_maps.append({
            "anchor": np.ascontiguousarray(anchor_tokens[:, sl]),
            "neigh": np.ascontiguousarray(neighbor_tokens[:, sl]),
            "wqT": wqT, "wkT": wkT, "wvT": wvT, "woT": woT,
            "biases": biases, "ones": np.ones((1, 128), np.float32),
        })
    return in_maps


def kernel(**inputs):
    nc = get_program()
    in_maps = make_in_maps(**inputs)
    res = run_bass_kernel_spmd(nc, in_maps, list(range(NCORES)))
    out = np.concatenate([res.results[c]["out"] for c in range(NCORES)],
                         axis=1)
    return out


# revision 10
# speedup vs baseline: 76599.9970x; 4.2238x over previous
"""Trainium2 Bass kernel for per-anchor local cross-attention.

Problem shapes (hardcoded per contract):
  anchor_tokens   [B=2, V=2048, D=512]
  neighbor_tokens [B=2, V=2048, K=32, D=512]
  Wq/Wk/Wv/Wo     [512, 512], bq/bk/bv/bo [512]
  out             [B, V, D] = attention(anchor over its K neighbors) @ Wo.T + bo

Sharding: V split across 8 cores (attention is local per anchor; weights
replicated). Each core handles Vs = 256 anchors for both batch entries.

Per-core plan (all on one NeuronCore, SPMD over 8):
  - Load anchor / neighbor token tiles naturally ([token, din]); PE-transpose
    128x128 blocks to get [din, token] stationary operands.
  - Projections run data-stationary on the PE: lhsT = X^T chunk [din,128tok],
    rhs = W^T chunk [din, 512 dout] -> PSUM [128 tok, 512 dout]; fp32r dtype
    (full-rate fp32 mode). Bias added via an extra ones-row matmul.
  - K/V token tiles are k-sliced (128 tokens = one neighbor index k0 across
    128 anchors), so projection outputs land directly in the attention-friendly
    layout Kt/Vt [anchor_p, k, h, d].
  - Attention on DVE/ACT: scores = reduce_d(Kt * Q_bcast) (scaled Wq on host),
    exp on ACT, sum/reciprocal, AV = reduce_k(Vt * w_bcast), final scale.
  - O-projection: PE-transpose attn -> data-stationary matmul -> DMA out.
"""

import numpy as np
from contextlib import ExitStack

import concourse.bass as bass
import concourse.tile as tile
from concourse import bacc, mybir
from concourse.bass_utils import run_bass_kernel_spmd
from concourse.masks import make_identity

# ---- problem constants ----
B = 2
V = 2048
D = 512
K = 32
H = 8
Dh = 64
NCORES = 8
VS = V // NCORES          # anchors per core
VT = 128                  # anchors per attention tile
N_VT = VS // VT           # vtiles per batch entry per core
DT = mybir.dt.float32
MMDT = mybir.dt.float32r  # matmul compute dtype (full-rate fp32 mode)
KQ = 4                    # k's per streaming chunk
N_KQ = K // KQ

_PROGRAM_CACHE = {}


RDT = mybir.dt.float32r  # dtype for matmul operand tiles (PE rounds on ingest)


def build_program_reps(reps=1):
    nc = bacc.Bacc("TRN2", target_bir_lowering=False, debug=False,
                   num_devices=NCORES)

    anchor = nc.dram_tensor("anchor", [B, VS, D], DT, kind="ExternalInput").ap()
    neigh = nc.dram_tensor("neigh", [B, VS, K, D], DT, kind="ExternalInput").ap()
    wqT = nc.dram_tensor("wqT", [D, D], RDT, kind="ExternalInput").ap()
    wkT = nc.dram_tensor("wkT", [D, D], RDT, kind="ExternalInput").ap()
    wvT = nc.dram_tensor("wvT", [D, D], RDT, kind="ExternalInput").ap()
    woT = nc.dram_tensor("woT", [D, D], RDT, kind="ExternalInput").ap()
    biases = nc.dram_tensor("biases", [4, D], RDT, kind="ExternalInput").ap()
    ones_d = nc.dram_tensor("ones", [1, 128], RDT, kind="ExternalInput").ap()
    out = nc.dram_tensor("out", [B, VS, D], DT, kind="ExternalOutput").ap()

    with tile.TileContext(nc) as tc, ExitStack() as ctx:
        const_pool = ctx.enter_context(tc.tile_pool(name="const", bufs=1))
        w_pool = ctx.enter_context(tc.tile_pool(name="weights", bufs=1))
        xnat_pool = ctx.enter_context(tc.tile_pool(name="xnat", bufs=2))
        xT_pool = ctx.enter_context(tc.tile_pool(name="xT", bufs=3))
        kt_pool = ctx.enter_context(tc.tile_pool(name="kt", bufs=2))
        q_pool = ctx.enter_context(tc.tile_pool(name="q", bufs=2))
        sc_pool = ctx.enter_context(tc.tile_pool(name="scores", bufs=2))
        prod_pool = ctx.enter_context(tc.tile_pool(name="prod", bufs=2))
        attn_pool = ctx.enter_context(tc.tile_pool(name="attn", bufs=2))
        y_pool = ctx.enter_context(tc.tile_pool(name="y", bufs=2))
        tps_pool = ctx.enter_context(
            tc.tile_pool(name="tpsum", bufs=2, space="PSUM"))
        mm_pool = ctx.enter_context(
            tc.tile_pool(name="mmpsum", bufs=2, space="PSUM"))

        # constants
        ident = const_pool.tile([128, 128], DT)
        make_identity(nc, ident[:])
        ones = const_pool.tile([1, 128], RDT)
        nc.sync.dma_start(ones[:], ones_d[:, :])

        # weights: [din(4x128 partition chunks), dout 512]
        wq_sb = w_pool.tile([128, 4, D], RDT)
        wk_sb = w_pool.tile([128, 4, D], RDT)
        wv_sb = w_pool.tile([128, 4, D], RDT)
        wo_sb = w_pool.tile([128, 4, D], RDT)
        for sb, dram in ((wq_sb, wqT), (wk_sb, wkT), (wv_sb, wvT), (wo_sb, woT)):
            for c in range(4):
                nc.sync.dma_start(sb[:, c], dram[c * 128:(c + 1) * 128, :])
        bias_sb = w_pool.tile([1, 4, D], RDT)
        nc.sync.dma_start(bias_sb[:, :, :], biases[:, :].unsqueeze(0))
        bq_sb, bk_sb, bv_sb, bo_sb = (bias_sb[:, i] for i in range(4))

        def transpose_128x512(src_view, dst_tile):
            """src [128, 512] -> dst SBUF [128, 4, 128] ([din_chunk, token])."""
            ps = tps_pool.tile([128, 4, 128], DT, tag="tps")
            for c in range(4):
                nc.tensor.transpose(ps[:, c], src_view[:, c * 128:(c + 1) * 128],
                                    ident[:])
            nc.scalar.copy(dst_tile[:], ps[:])

        def project(xT, w_sb, b_sb, ps):
            """PSUM [128tok, 512] = xT.T @ W^T + ones.T @ bias."""
            for c in range(4):
                nc.tensor.matmul(ps[:], xT[:, c], w_sb[:, c],
                                 start=(c == 0), stop=False)
            nc.tensor.matmul(ps[:], ones[:1, :], b_sb[:1, :],
                             start=False, stop=True)

        for _rep in range(reps):
          for b in range(B):
            for vt in range(N_VT):
                v0 = vt * VT
                # ---- Q projection ----
                q_nat = xnat_pool.tile([128, D], DT, tag="xnat")
                nc.sync.dma_start(q_nat[:], anchor[b, v0:v0 + VT, :])
                qT = xT_pool.tile([128, 4, 128], RDT, tag="qaT", bufs=2)
                transpose_128x512(q_nat[:], qT)
                q_ps = mm_pool.tile([128, D], DT, tag="qy", bufs=1)
                project(qT, wq_sb, bq_sb, q_ps)
                q_sb = q_pool.tile([128, H, Dh], DT)
                nc.scalar.copy(q_sb[:], q_ps[:])

                # ---- streamed K/V projections + online-softmax attention ----
                ssum_acc = None
                av_acc = None
                for kq in range(N_KQ):
                    ktq = kt_pool.tile([128, KQ, H, Dh], DT, tag="ktq", bufs=3)
                    vtq = kt_pool.tile([128, KQ, H, Dh], DT, tag="vtq", bufs=3)
                    x4 = xnat_pool.tile([128, KQ, D], DT, tag="xnat")
                    k0 = kq * KQ
                    nc.sync.dma_start(
                        x4[:], neigh[b, v0:v0 + VT, k0:k0 + KQ, :])
                    for jj in range(KQ):
                        xT = xT_pool.tile([128, 4, 128], RDT, tag="xT")
                        transpose_128x512(x4[:, jj], xT)
                        k_ps = mm_pool.tile([128, D], DT, tag="kps", bufs=3)
                        project(xT, wk_sb, bk_sb, k_ps)
                        v_ps = mm_pool.tile([128, D], DT, tag="vps")
                        project(xT, wv_sb, bv_sb, v_ps)
                        nc.scalar.copy(ktq[:, jj], k_ps[:])
                        nc.scalar.copy(vtq[:, jj], v_ps[:])
                    # scores chunk: prod[kj, h, d] = Kt * Q, reduce over d
                    prod = prod_pool.tile([128, KQ, H, Dh], DT, tag="prod",
                                          bufs=3)
                    q_b = q_sb[:].unsqueeze(1).broadcast_to([128, KQ, H, Dh])
                    nc.vector.tensor_tensor(
                        out=prod[:], in0=ktq[:], in1=q_b,
                        op=mybir.AluOpType.mult)
                    scq = sc_pool.tile([128, H, KQ], DT, tag="scq", bufs=2)
                    nc.vector.tensor_reduce(
                        out=scq[:].transpose([0, 2, 1]), in_=prod[:],
                        axis=mybir.AxisListType.X, op=mybir.AluOpType.add)
                    # exp (no max-sub: |scores| <~ 6 is fp32-safe)
                    wq_t = sc_pool.tile([128, H, KQ], DT, tag="wq", bufs=2)
                    nc.scalar.activation(wq_t[:], scq[:],
                                         mybir.ActivationFunctionType.Exp)
                    sp = sc_pool.tile([128, H], DT, tag="sp", bufs=2)
                    nc.vector.tensor_reduce(
                        out=sp[:], in_=wq_t[:], axis=mybir.AxisListType.X,
                        op=mybir.AluOpType.add)
                    if ssum_acc is None:
                        ssum_acc = sp
                    else:
                        nsa = sc_pool.tile([128, H], DT, tag="ssacc", bufs=2)
                        nc.vector.tensor_add(nsa[:], ssum_acc[:], sp[:])
                        ssum_acc = nsa
                    # AV partial: prod2[h, d, kj] = V * w (GPSIMD), reduce kj
                    prod2 = prod_pool.tile([128, H, Dh, KQ], DT, tag="prod",
                                           bufs=3)
                    v_view = vtq[:].transpose([0, 2, 3, 1])
                    w_view = wq_t[:].unsqueeze(2).broadcast_to(
                        [128, H, Dh, KQ])
                    nc.gpsimd.tensor_tensor(
                        out=prod2[:], in0=v_view, in1=w_view,
                        op=mybir.AluOpType.mult)
                    part = attn_pool.tile([128, H, Dh], DT, tag="avp", bufs=2)
                    nc.vector.tensor_reduce(
                        out=part[:], in_=prod2[:], axis=mybir.AxisListType.X,
                        op=mybir.AluOpType.add)
                    if av_acc is None:
                        av_acc = part
                    else:
                        nxt = attn_pool.tile([128, H, Dh], DT, tag="avacc",
                                             bufs=2)
                        nc.vector.tensor_add(nxt[:], av_acc[:], part[:])
                        av_acc = nxt
                # normalize
                rec = sc_pool.tile([128, H], DT, tag="rec", bufs=2)
                nc.vector.reciprocal(rec[:], ssum_acc[:])
                attn = attn_pool.tile([128, H, Dh], DT, tag="attn", bufs=2)
                rec_b = rec[:].unsqueeze(2).broadcast_to([128, H, Dh])
                nc.vector.tensor_tensor(out=attn[:], in0=av_acc[:], in1=rec_b,
                                        op=mybir.AluOpType.mult)

                # ---- O projection ----
                attn_flat = attn[:].rearrange("p h d -> p (h d)")
                aT = xT_pool.tile([128, 4, 128], RDT, tag="qaT", bufs=2)
                transpose_128x512(attn_flat, aT)
                y_ps = mm_pool.tile([128, D], DT, tag="qy", bufs=1)
                project(aT, wo_sb, bo_sb, y_ps)
                y_sb = y_pool.tile([128, D], DT)
                nc.scalar.copy(y_sb[:], y_ps[:])
                nc.sync.dma_start(out[b, v0:v0 + VT, :], y_sb[:])

    nc.compile()
    return nc


def build_program():
    return build_program_reps(1)


def get_program():
    if "nc" not in _PROGRAM_CACHE:
        _PROGRAM_CACHE["nc"] = build_program()
    return _PROGRAM_CACHE["nc"]


def make_in_maps(anchor_tokens, neighbor_tokens, Wq, bq, Wk, bk, Wv, bv, Wo, bo):
    scale = np.float32(1.0 / np.sqrt(Dh))
    wqT = np.ascontiguousarray(Wq.T * scale, dtype=np.float32)
    wkT = np.ascontiguousarray(Wk.T, dtype=np.float32)
    wvT = np.ascontiguousarray(Wv.T, dtype=np.float32)
    woT = np.ascontiguousarray(Wo.T, dtype=np.float32)
    biases = np.stack([bq * scale, bk, bv, bo]).astype(np.float32)
    anchor_tokens = np.asarray(anchor_tokens, dtype=np.float32)
    neighbor_tokens = np.asarray(neighbor_tokens, dtype=np.float32)
    in_maps = []
    for c in range(NCORES):
        sl = slice(c * VS, (c + 1) * VS)
        in_maps.append({
            "anchor": np.ascontiguousarray(anchor_tokens[:, sl]),
            "neigh": np.ascontiguousarray(neighbor_tokens[:, sl]),
            "wqT": wqT, "wkT": wkT, "wvT": wvT, "woT": woT,
            "biases": biases, "ones": np.ones((1, 128), np.float32),
        })
    return in_maps


def kernel(**inputs):
    nc = get_program()
    in_maps = make_in_maps(**inputs)
    res = run_bass_kernel_spmd(nc, in_maps, list(range(NCORES)))
    out = np.concatenate([res.results[c]["out"] for c in range(NCORES)],
                         axis=1)
    return out
